# revision 34
# baseline (speedup 1.0000x reference)
"""MXFP4-quantized linear kernel for Trainium2 (8 NeuronCores, SPMD).

Problem: out = quant_mxfp4(x) @ W.T + bias
  x [2, 4096, 4096] f32, W [11008, 4096] f32, bias [11008] f32 -> out [2, 4096, 11008] f32

Strategy (data-parallel over rows of x):
  - Host: flatten x to [8192, 4096], shard rows 8 ways; pre-transpose W to
    WT [4096, 11008] fp16 (static weight preprocessing).
  - Each core: quantize its x shard (dynamic per-32-block MXFP4) on-chip,
    then dense fp16 GEMM (fp32 PSUM accumulate) against streamed WT tiles.

Quant pipeline (per 128x512 chunk) — unified offset-window snap:
  amax  = blockwise max|x|                      (DVE reduce, f32)
  sc16  = fp16(amax/6)  (ACT)    r2 = 1/sc16    (DVE reciprocal, f32)
  w     = x * r2                                 (GPSIMD, f32)
  pcb   = w & 0x7f800000      exponent extract   (DVE tensor_scalar, u32)
  o     = max(pcb,1.0) * (1.5*2^22)              (DVE tensor_scalar chain)
  t     = w + o   -> single f32 RNE = snap-to-grid-in-window (GPSIMD)
  s     = (t - o) -> f16  exact, on the MXFP4 grid scaled by 1 (DVE)
  xqc   = s * sc16 -> f16                        (DVE)
  transpose xqc into K-major xqT via DMA xbar (dma_start_transpose)
The offset window o = 768*2^max(e,0) in f32 space (CR*pc) makes one
rounded add implement RNE onto {0,.5,1,1.5,2,3,4,6} for all |w|<8, signs
included. Ties (exact fp midpoints) go to-even vs reference to-lower:
measure-zero on continuous inputs.

GEMM: early phase (first EARLY_NC n-chunks) is emitted per-m-tile one
m-tile behind quant so the PE consumes tiles as they are produced; bias
for early blocks rides a K=1 ones x bias16 matmul that opens each PSUM
accumulation group, so early drains are pure ACT copies (quant engines
stay unblocked).  Steady state streams the remaining n-chunks in waves
of 4 m-tiles over 8 PSUM banks; bias added during the DVE drain.
"""
import sys

try:
    import concourse  # noqa: F401
except ImportError:
    sys.path.insert(0, "/opt/trn_rl_repo")

import numpy as np

import concourse.bacc as bacc
import concourse.mybir as mybir
from concourse import tile
from concourse.masks import make_identity
from concourse.bass_utils import run_bass_kernel_spmd

F32, F16 = mybir.dt.float32, mybir.dt.float16
U32 = mybir.dt.uint32
ACT = mybir.ActivationFunctionType
ALU = mybir.AluOpType

CR = float(1.5 * 2**22)    # f32 window: ulp 0.5 at [1.5*2^22, 3*2^22)
EXPMASK = 0x7F800000       # f32 exponent field
ONEBITS = 0x3F800000       # bits of 1.0f

N_CORES = 8
B, S, K, N = 2, 4096, 4096, 11008
M = B * S                  # 8192
MS = M // N_CORES          # 1024 rows per core
QC = 512                   # quant chunk width (along K)


def build_program(Ms=MS, Kd=K, Nd=N, early_widths=(512, 512, 384),
                  wt_extra=0, transpose_mode="pe"):
    """Build the SPMD Bass program for one core (same program on all cores)."""
    nc = bacc.Bacc("TRN2", target_bir_lowering=False, debug=False)
    x = nc.dram_tensor("x", [Ms, Kd], F32, kind="ExternalInput")
    wt = nc.dram_tensor("wt", [Kd, Nd], F16, kind="ExternalInput")
    bias = nc.dram_tensor("bias", [Nd], F32, kind="ExternalInput")
    out = nc.dram_tensor("out", [Ms, Nd], F32, kind="ExternalOutput")

    MT = Ms // 128          # m-tiles per core
    KT = Kd // 128          # k-tiles
    NB = QC // 32           # quant blocks per chunk
    QCH = Kd // QC          # quant chunks per m-tile
    KB = Kd // 32           # amax blocks per m-tile
    TPC = QC // 128         # transposes per chunk (pe mode)

    early_nc = len(early_widths)
    nchunks = []
    n0 = 0
    for wdt in early_widths:
        nchunks.append((n0, wdt))
        n0 += wdt
    while n0 < Nd:
        nw = min(512, Nd - n0)
        nchunks.append((n0, nw))
        n0 += nw
    # wt group-tiles that must be simultaneously live: early set + streaming
    wt_bufs = len(early_widths) * (KT // 2) + wt_extra

    with tile.TileContext(nc) as tc:
        with (
            tc.tile_pool(name="xqt", bufs=1) as xqt_pool,
            tc.tile_pool(name="xin", bufs=6) as xin_pool,
            tc.tile_pool(name="qw", bufs=2) as qw_pool,
            tc.tile_pool(name="qpc", bufs=1) as qpc_pool,
            tc.tile_pool(name="qo", bufs=3) as qo_pool,
            tc.tile_pool(name="qt", bufs=2) as qt_pool,
            tc.tile_pool(name="qs", bufs=4) as qs_pool,
            tc.tile_pool(name="qsmall", bufs=4) as qsmall_pool,
            tc.tile_pool(name="wtp", bufs=wt_bufs) as wt_pool,
            tc.tile_pool(name="outp", bufs=2) as out_pool,
            tc.tile_pool(name="bnc", bufs=1) as bias_pool,
            tc.tile_pool(name="cst", bufs=1) as const_pool,
            tc.tile_pool(name="psum", bufs=8 if transpose_mode == "dma" else 6,
                         space="PSUM") as psum_pool,
            tc.tile_pool(name="psumt", bufs=2, space="PSUM") as psumt_pool,
        ):
            ident = const_pool.tile([128, 128], F16, tag="ident")
            make_identity(nc, ident[:])

            ones_row = const_pool.tile([1, 128], F16, tag="ones")
            nc.vector.memset(ones_row[:], 1.0)

            # persistent K-major quantized activations: [128, MT*Kd] f16;
            # (mt, k) tile at cols mt*Kd + k*128, col index = m-within-tile
            xqT = xqt_pool.tile([128, MT * Kd], F16, tag="xqT")

            def lhsT(k, mt):
                return xqT[:, mt * Kd + k * 128: mt * Kd + (k + 1) * 128]

            # ---- weight loads: 2 k-tiles per DMA (one 3D descriptor) —
            # amortizes the ~0.6us per-issue engine cost while keeping each
            # push small enough not to stall the issuing engine on
            # descriptor-ring space.
            GK = 2                       # k-tiles per group
            NG = KT // GK                # groups per n-chunk

            def wt_group_issues(nci, n0, nw, tag=""):
                """Return ([issue closures], wts list of per-k AP views)."""
                wts = [None] * KT
                issues = []
                for g in range(NG):
                    wtt = wt_pool.tile([128, GK * nw], F16, tag="wt",
                                       name=f"wt{tag}{nci}_{g}")
                    for j in range(GK):
                        wts[g * GK + j] = wtt[:, j * nw:(j + 1) * nw]
                    src = wt[g * GK * 128:(g + 1) * GK * 128, n0:n0 + nw]
                    src = src.rearrange("(j p) n -> p j n", p=128)
                    dst = wtt[:].rearrange("p (j n) -> p j n", j=GK)

                    def mk(eng, dst=dst, src=src):
                        return lambda: eng.dma_start(out=dst, in_=src)
                    issues.append(mk(nc.sync if g % 2 == 0 else nc.scalar))
                return issues, wts

            early = []
            early_wt_issues = []
            for nci in range(early_nc):
                n0, nw = nchunks[nci]
                b32row = bias_pool.tile([1, nw], F32, tag="b32row", bufs=1,
                                        name=f"b32r{nci}")
                nc.sync.dma_start(out=b32row[:], in_=bias[n0:n0 + nw].unsqueeze(0))
                b16row = bias_pool.tile([1, nw], F16, tag="b16row", bufs=3,
                                        name=f"b16r{nci}")
                nc.scalar.copy(out=b16row[:], in_=b32row[:])
                issues, wts = wt_group_issues(nci, n0, nw)
                early_wt_issues.extend(issues)
                early.append((nci, n0, nw, wts, b16row))

            # ---- Phase A: quantize + (one m-tile behind) early GEMM ----
            def pop_wt_issues(n):
                for _ in range(min(n, len(early_wt_issues))):
                    early_wt_issues.pop(0)()

            # quant compute (DVE/GPS only): per-chunk ops software-pipelined
            # by one chunk (pcb/o of chunk q emitted before s/xqc of q-1) so
            # GPSIMD's t(q) never waits on a DVE op that sits behind s/xqc in
            # the DVE FIFO. Scales are batched per half m-tile so only 4+2
            # xin chunks need to stay live.
            xqcs = {}

            def emit_quant(mt, xin_eng=None):
                xin_eng = xin_eng or nc.sync
                HQ = QCH // 2            # chunks per half
                HB = KB // 2             # amax blocks per half
                scale = {}
                xins = [None] * QCH
                deferred = None
                xqcs[mt] = [None] * QCH

                def flush_deferred():
                    nonlocal deferred
                    if deferred is None:
                        return
                    q, t, o, sc16 = deferred
                    s = qs_pool.tile([128, QC], F16, tag="qs", name=f"s{mt}_{q}")
                    nc.vector.tensor_tensor(out=s[:], in0=t[:], in1=o[:],
                                            op=ALU.subtract)
                    nc.vector.tensor_tensor(
                        out=s.rearrange("p (b c) -> p b c", c=32),
                        in0=s.rearrange("p (b c) -> p b c", c=32),
                        in1=sc16.unsqueeze(2).broadcast_to([128, NB, 32]),
                        op=ALU.mult)
                    xqcs[mt][q] = s
                    deferred = None

                for q in range(QCH):
                    if q % HQ == 0:      # start of a half: reduce + scales
                        h = q // HQ
                        amax_h = qsmall_pool.tile([128, HB], F32, tag="amax",
                                                  bufs=4, name=f"amax{mt}_{h}")
                        sc16_h = qsmall_pool.tile([128, HB], F16, tag="sc16",
                                                  bufs=4, name=f"sc16{mt}_{h}")
                        r2_h = qsmall_pool.tile([128, HB], F32, tag="r2",
                                                bufs=4, name=f"r2{mt}_{h}")
                        for qq in range(q, q + HQ):
                            xin = xin_pool.tile([128, QC], F32, tag="xin",
                                                name=f"xin{mt}_{qq}")
                            xin_eng.dma_start(
                                out=xin[:],
                                in_=x[mt * 128:(mt + 1) * 128, qq * QC:(qq + 1) * QC])
                            pop_wt_issues(2)
                            nc.vector.tensor_reduce(
                                out=amax_h[:, (qq - q) * NB:(qq - q + 1) * NB],
                                in_=xin.rearrange("p (b c) -> p b c", c=32),
                                axis=mybir.AxisListType.X, op=ALU.max,
                                apply_absolute_value=True)
                            xins[qq] = xin
                        nc.scalar.activation(out=sc16_h[:], in_=amax_h[:],
                                             func=ACT.Copy, scale=float(1.0 / 6.0))
                        nc.vector.reciprocal(out=r2_h[:], in_=sc16_h[:])
                        scale[h] = (sc16_h, r2_h)

                    h = q // HQ
                    qh = q - h * HQ
                    sc16_h, r2_h = scale[h]
                    r2 = r2_h[:, qh * NB:(qh + 1) * NB]
                    sc16 = sc16_h[:, qh * NB:(qh + 1) * NB]
                    xin = xins[q]

                    w = qw_pool.tile([128, QC], F32, tag="qw", name=f"w{mt}_{q}")
                    nc.gpsimd.tensor_tensor(
                        out=w.rearrange("p (b c) -> p b c", c=32),
                        in0=xin.rearrange("p (b c) -> p b c", c=32),
                        in1=r2.unsqueeze(2).broadcast_to([128, NB, 32]),
                        op=ALU.mult)
                    pcb = qpc_pool.tile([128, QC], U32, tag="qpc", name=f"pcb{mt}_{q}")
                    nc.vector.tensor_scalar(out=pcb[:], in0=w[:].bitcast(U32),
                                            scalar1=EXPMASK, scalar2=None,
                                            op0=ALU.bitwise_and)
                    o = qo_pool.tile([128, QC], F32, tag="qo", name=f"o{mt}_{q}")
                    nc.vector.tensor_scalar(out=o[:], in0=pcb[:].bitcast(F32),
                                            scalar1=1.0, scalar2=CR,
                                            op0=ALU.max, op1=ALU.mult)
                    t = qt_pool.tile([128, QC], F32, tag="qt", name=f"t{mt}_{q}")
                    nc.gpsimd.tensor_tensor(out=t[:], in0=w[:], in1=o[:], op=ALU.add)
                    flush_deferred()
                    deferred = (q, t, o, sc16)
                flush_deferred()

            def emit_transposes(mt, qlist):
                for q in qlist:
                    xqc = xqcs[mt][q]
                    dst = xqT[:, mt * Kd + q * QC: mt * Kd + (q + 1) * QC]
                    pt = psumt_pool.tile([128, QC], F16, tag="tp",
                                         name=f"pt{mt}_{q}")
                    for j in range(TPC):
                        nc.tensor.transpose(pt[:, j * 128:(j + 1) * 128],
                                            xqc[:, j * 128:(j + 1) * 128],
                                            ident[:])
                    nc.scalar.copy(out=dst, in_=pt[:])

            # early GEMM MMs (bias pre-injected via K=1 matmul) and their
            # drains are split: drains are emitted TWO m-tiles behind so the
            # ACT copy (which waits on the early MMs) never head-of-line
            # blocks the next m-tile's sc16 / transpose copies on ACT.
            early_psums = {}

            def emit_early_block(mt, b):
                nci, n0, nw, wts, b16row = early[b]
                ps = psum_pool.tile([128, nw], F32, tag="ps", name=f"ps{nci}_{mt}")
                nc.tensor.matmul(out=ps[:], lhsT=ones_row[:], rhs=b16row[:],
                                 start=True, stop=False)
                for k in range(KT):
                    nc.tensor.matmul(out=ps[:], lhsT=lhsT(k, mt), rhs=wts[k][:],
                                     start=False, stop=(k == KT - 1))
                early_psums.setdefault(mt, []).append((nci, n0, nw, ps))

            def emit_early_drains(mt):
                for nci, n0, nw, ps in early_psums.pop(mt):
                    ot = out_pool.tile([128, nw], F32, tag="ot", name=f"ot{nci}_{mt}")
                    nc.scalar.copy(out=ot[:], in_=ps[:])
                    nc.sync.dma_start(out=out[mt * 128:(mt + 1) * 128, n0:n0 + nw],
                                      in_=ot[:])

            # Two m-half passes: only m-tiles 0..3 need the (slower) early
            # phase; m-tiles 4..7 are quantized in the background during
            # pass 1 and processed entirely at the full steady rate in
            # pass 2. W is streamed twice — DMA has the headroom.
            MH = MT // 2

            # PE order per period: [E(mt-1, b) | T(mt, chunk group b)] so the
            # PE alternates early-GEMM blocks with short transpose bursts and
            # the xqc ring drains steadily.
            tgroups = [(0, 1, 2), (3, 4, 5), (6, 7)]
            for mt in range(MH):
                emit_quant(mt)
                # all wt writers must be emitted before their first reader
                pop_wt_issues(len(early_wt_issues))
                if mt >= 2:
                    emit_early_drains(mt - 2)
                if mt == 0:
                    emit_transposes(0, range(QCH))
                else:
                    for b in range(early_nc):
                        emit_early_block(mt - 1, b)
                        emit_transposes(mt, tgroups[b])
            for b in range(early_nc):
                emit_early_block(MH - 1, b)
            emit_early_drains(MH - 2)
            emit_early_drains(MH - 1)

            # quantize the second m-half; transposes are sprinkled into the
            # first GEMM pass below so the PE never waits on them. xin loads
            # issue from GPSIMD so their slot-waits (paced by those
            # transposes) never head-of-line block the sync queue's weight
            # streams for pass 1.
            for mt in range(MH, MT):
                emit_quant(mt, xin_eng=nc.gpsimd)

            # ---- GEMM passes ----
            def load_bias(nci, n0, nw, tag):
                bnc = bias_pool.tile([128, nw], F32, tag="bnc", name=f"bnc{tag}{nci}")
                nc.sync.dma_start(
                    out=bnc[:],
                    in_=bias[n0:n0 + nw].unsqueeze(0).broadcast_to([128, nw]))
                return bnc

            def emit_nchunk(nci, n0, nw, mts, act_drain, tag):
                issues, wts = wt_group_issues(nci, n0, nw, tag)
                for iss in issues:
                    iss()
                if act_drain:
                    b32 = bias_pool.tile([1, nw], F32, tag="b32row", bufs=1,
                                         name=f"b32{tag}{nci}")
                    nc.sync.dma_start(out=b32[:], in_=bias[n0:n0 + nw].unsqueeze(0))
                    b16 = bias_pool.tile([1, nw], F16, tag="b16row", bufs=3,
                                         name=f"b16{tag}{nci}")
                    nc.scalar.copy(out=b16[:], in_=b32[:])
                else:
                    bnc = load_bias(nci, n0, nw, tag)
                psums = []
                for mt in mts:
                    ps = psum_pool.tile([128, nw], F32, tag="ps",
                                        name=f"ps{tag}{nci}_{mt}")
                    if act_drain:
                        nc.tensor.matmul(out=ps[:], lhsT=ones_row[:], rhs=b16[:],
                                         start=True, stop=False)
                    psums.append(ps)
                for k in range(KT):
                    for j, mt in enumerate(mts):
                        nc.tensor.matmul(
                            out=psums[j][:], lhsT=lhsT(k, mt), rhs=wts[k][:],
                            start=(not act_drain and k == 0), stop=(k == KT - 1))
                for j, mt in enumerate(mts):
                    ot = out_pool.tile([128, nw], F32, tag="ot",
                                       name=f"ot{tag}{nci}_{mt}")
                    if act_drain:
                        nc.scalar.copy(out=ot[:], in_=psums[j][:])
                    else:
                        nc.vector.tensor_tensor(out=ot[:], in0=psums[j][:],
                                                in1=bnc[:, :nw], op=ALU.add)
                    nc.sync.dma_start(
                        out=out[mt * 128:(mt + 1) * 128, n0:n0 + nw], in_=ot[:])

            # pass 1: remaining n-chunks over m0..3. ACT drains for the first
            # few n-chunks — the DVE is still busy quantizing m4..7 then.
            mts1 = list(range(MH))
            for idx, nci in enumerate(range(early_nc, len(nchunks))):
                n0, nw = nchunks[nci]
                emit_nchunk(nci, n0, nw, mts1, act_drain=(idx < 6), tag="a")
                if idx < MT - MH:
                    emit_transposes(MH + idx, range(QCH))
            # pass 2: ALL n-chunks over m4..7 at full steady rate.
            mts2 = list(range(MH, MT))
            for nci in range(len(nchunks)):
                n0, nw = nchunks[nci]
                emit_nchunk(nci, n0, nw, mts2, act_drain=False, tag="b")
    nc.compile()
    return nc


_CACHE = {}


def _get_program():
    if "nc" not in _CACHE:
        _CACHE["nc"] = build_program()
    return _CACHE["nc"]


def run(x, W, bias, trace=False):
    nc = _get_program()
    xf = np.ascontiguousarray(np.asarray(x, dtype=np.float32).reshape(M, K))
    WT16 = np.ascontiguousarray(np.asarray(W, dtype=np.float32).T.astype(np.float16))
    b32 = np.ascontiguousarray(np.asarray(bias, dtype=np.float32))
    in_maps = [
        {"x": xf[c * MS:(c + 1) * MS], "wt": WT16, "bias": b32}
        for c in range(N_CORES)
    ]
    res = run_bass_kernel_spmd(nc, in_maps, list(range(N_CORES)), trace=trace)
    outs = [res.results[c]["out"] for c in range(N_CORES)]
    full = np.concatenate(outs, axis=0).reshape(B, S, N)
    return full, res


def kernel(x, W, bias):
    out, _ = run(x, W, bias, trace=False)
    return out


# revision 37
# speedup vs baseline: 1.1531x; 1.1531x over previous
"""MXFP4-quantized linear kernel for Trainium2 (8 NeuronCores, SPMD).

Problem: out = quant_mxfp4(x) @ W.T + bias
  x [2, 4096, 4096] f32, W [11008, 4096] f32, bias [11008] f32 -> out [2, 4096, 11008] f32

Strategy (data-parallel over rows of x):
  - Host: flatten x to [8192, 4096], shard rows 8 ways; pre-transpose W to
    WT [4096, 11008] fp16 (static weight preprocessing).
  - Each core: quantize its x shard (dynamic per-32-block MXFP4) on-chip,
    then dense fp16 GEMM (fp32 PSUM accumulate) against streamed WT tiles.

Quant pipeline (per 128x512 chunk) — unified offset-window snap:
  amax  = blockwise max|x|                      (DVE reduce, f32)
  sc16  = fp16(amax/6)  (ACT)    r2 = 1/sc16    (DVE reciprocal, f32)
  w     = x * r2                                 (GPSIMD, f32)
  pcb   = w & 0x7f800000      exponent extract   (DVE tensor_scalar, u32)
  o     = max(pcb,1.0) * (1.5*2^22)              (DVE tensor_scalar chain)
  t     = w + o   -> single f32 RNE = snap-to-grid-in-window (GPSIMD)
  s     = (t - o) -> f16  exact, on the MXFP4 grid scaled by 1 (DVE)
  xqc   = s * sc16 -> f16                        (DVE)
  transpose xqc into K-major xqT via DMA xbar (dma_start_transpose)
The offset window o = 768*2^max(e,0) in f32 space (CR*pc) makes one
rounded add implement RNE onto {0,.5,1,1.5,2,3,4,6} for all |w|<8, signs
included. Ties (exact fp midpoints) go to-even vs reference to-lower:
measure-zero on continuous inputs.

GEMM: early phase (first EARLY_NC n-chunks) is emitted per-m-tile one
m-tile behind quant so the PE consumes tiles as they are produced; bias
for early blocks rides a K=1 ones x bias16 matmul that opens each PSUM
accumulation group, so early drains are pure ACT copies (quant engines
stay unblocked).  Steady state streams the remaining n-chunks in waves
of 4 m-tiles over 8 PSUM banks; bias added during the DVE drain.
"""
import sys

try:
    import concourse  # noqa: F401
except ImportError:
    sys.path.insert(0, "/opt/trn_rl_repo")

import numpy as np

import concourse.bacc as bacc
import concourse.mybir as mybir
from concourse import tile
from concourse.masks import make_identity
from concourse.bass_utils import run_bass_kernel_spmd

F32, F16 = mybir.dt.float32, mybir.dt.float16
U32 = mybir.dt.uint32
ACT = mybir.ActivationFunctionType
ALU = mybir.AluOpType

CR = float(1.5 * 2**22)    # f32 window: ulp 0.5 at [1.5*2^22, 3*2^22)
EXPMASK = 0x7F800000       # f32 exponent field
ONEBITS = 0x3F800000       # bits of 1.0f

N_CORES = 8
B, S, K, N = 2, 4096, 4096, 11008
M = B * S                  # 8192
MS = M // N_CORES          # 1024 rows per core
QC = 512                   # quant chunk width (along K)


def build_program(Ms=MS, Kd=K, Nd=N, early_widths=(512, 512),
                  wt_extra=0, transpose_mode="pe"):
    """Build the SPMD Bass program for one core (same program on all cores)."""
    nc = bacc.Bacc("TRN2", target_bir_lowering=False, debug=False)
    x = nc.dram_tensor("x", [Ms, Kd], F32, kind="ExternalInput")
    wt = nc.dram_tensor("wt", [Kd, Nd], F16, kind="ExternalInput")
    bias = nc.dram_tensor("bias", [Nd], F32, kind="ExternalInput")
    out = nc.dram_tensor("out", [Ms, Nd], F32, kind="ExternalOutput")

    MT = Ms // 128          # m-tiles per core
    KT = Kd // 128          # k-tiles
    NB = QC // 32           # quant blocks per chunk
    QCH = Kd // QC          # quant chunks per m-tile
    KB = Kd // 32           # amax blocks per m-tile
    TPC = QC // 128         # transposes per chunk (pe mode)

    early_nc = len(early_widths)
    nchunks = []
    n0 = 0
    for wdt in early_widths:
        nchunks.append((n0, wdt))
        n0 += wdt
    while n0 < Nd:
        nw = min(512, Nd - n0)
        nchunks.append((n0, nw))
        n0 += nw
    # wt group-tiles that must be simultaneously live: early set + streaming
    wt_bufs = len(early_widths) * (KT // 2) + wt_extra

    with tile.TileContext(nc) as tc:
        with (
            tc.tile_pool(name="xqt", bufs=1) as xqt_pool,
            tc.tile_pool(name="xin", bufs=8) as xin_pool,
            tc.tile_pool(name="qw", bufs=3) as qw_pool,
            tc.tile_pool(name="qpc", bufs=2) as qpc_pool,
            tc.tile_pool(name="qo", bufs=3) as qo_pool,
            tc.tile_pool(name="qt", bufs=3) as qt_pool,
            tc.tile_pool(name="qs", bufs=5) as qs_pool,
            tc.tile_pool(name="qsmall", bufs=4) as qsmall_pool,
            tc.tile_pool(name="wtp", bufs=wt_bufs) as wt_pool,
            tc.tile_pool(name="outp", bufs=2) as out_pool,
            tc.tile_pool(name="bnc", bufs=1) as bias_pool,
            tc.tile_pool(name="cst", bufs=1) as const_pool,
            tc.tile_pool(name="psum", bufs=8 if transpose_mode == "dma" else 6,
                         space="PSUM") as psum_pool,
            tc.tile_pool(name="psumt", bufs=2, space="PSUM") as psumt_pool,
        ):
            ident = const_pool.tile([128, 128], F16, tag="ident")
            make_identity(nc, ident[:])

            ones_row = const_pool.tile([1, 128], F16, tag="ones")
            nc.vector.memset(ones_row[:], 1.0)

            # persistent K-major quantized activations: [128, MT*Kd] f16;
            # (mt, k) tile at cols mt*Kd + k*128, col index = m-within-tile
            xqT = xqt_pool.tile([128, MT * Kd], F16, tag="xqT")

            def lhsT(k, mt):
                return xqT[:, mt * Kd + k * 128: mt * Kd + (k + 1) * 128]

            # ---- weight loads: 2 k-tiles per DMA (one 3D descriptor) —
            # amortizes the ~0.6us per-issue engine cost while keeping each
            # push small enough not to stall the issuing engine on
            # descriptor-ring space.
            GK = 2                       # k-tiles per group
            NG = KT // GK                # groups per n-chunk

            def wt_group_issues(nci, n0, nw, tag=""):
                """Return ([issue closures], wts list of per-k AP views)."""
                wts = [None] * KT
                issues = []
                for g in range(NG):
                    wtt = wt_pool.tile([128, GK * nw], F16, tag="wt",
                                       name=f"wt{tag}{nci}_{g}")
                    for j in range(GK):
                        wts[g * GK + j] = wtt[:, j * nw:(j + 1) * nw]
                    src = wt[g * GK * 128:(g + 1) * GK * 128, n0:n0 + nw]
                    src = src.rearrange("(j p) n -> p j n", p=128)
                    dst = wtt[:].rearrange("p (j n) -> p j n", j=GK)

                    def mk(eng, dst=dst, src=src):
                        return lambda: eng.dma_start(out=dst, in_=src)
                    issues.append(mk(nc.sync if g % 2 == 0 else nc.scalar))
                return issues, wts

            early = []
            early_wt_issues = []
            for nci in range(early_nc):
                n0, nw = nchunks[nci]
                b32row = bias_pool.tile([1, nw], F32, tag="b32row", bufs=1,
                                        name=f"b32r{nci}")
                nc.sync.dma_start(out=b32row[:], in_=bias[n0:n0 + nw].unsqueeze(0))
                b16row = bias_pool.tile([1, nw], F16, tag="b16row", bufs=3,
                                        name=f"b16r{nci}")
                nc.scalar.copy(out=b16row[:], in_=b32row[:])
                issues, wts = wt_group_issues(nci, n0, nw)
                early_wt_issues.extend(issues)
                early.append((nci, n0, nw, wts, b16row))

            # ---- Phase A: quantize + (one m-tile behind) early GEMM ----
            def pop_wt_issues(n):
                for _ in range(min(n, len(early_wt_issues))):
                    early_wt_issues.pop(0)()

            # quant compute (DVE/GPS only): per-chunk ops software-pipelined
            # by one chunk (pcb/o of chunk q emitted before s/xqc of q-1) so
            # GPSIMD's t(q) never waits on a DVE op that sits behind s/xqc in
            # the DVE FIFO. Scales are batched per half m-tile so only 4+2
            # xin chunks need to stay live.
            xqcs = {}

            def emit_quant(mt, xin_eng=None):
                xin_eng = xin_eng or nc.sync
                HQ = QCH // 2            # chunks per half
                HB = KB // 2             # amax blocks per half
                scale = {}
                xins = [None] * QCH
                deferred = None
                xqcs[mt] = [None] * QCH

                def flush_deferred():
                    nonlocal deferred
                    if deferred is None:
                        return
                    q, t, o, sc16 = deferred
                    s = qs_pool.tile([128, QC], F16, tag="qs", name=f"s{mt}_{q}")
                    nc.vector.tensor_tensor(out=s[:], in0=t[:], in1=o[:],
                                            op=ALU.subtract)
                    nc.vector.tensor_tensor(
                        out=s.rearrange("p (b c) -> p b c", c=32),
                        in0=s.rearrange("p (b c) -> p b c", c=32),
                        in1=sc16.unsqueeze(2).broadcast_to([128, NB, 32]),
                        op=ALU.mult)
                    xqcs[mt][q] = s
                    deferred = None

                for q in range(QCH):
                    if q % HQ == 0:      # start of a half: reduce + scales
                        h = q // HQ
                        amax_h = qsmall_pool.tile([128, HB], F32, tag="amax",
                                                  bufs=4, name=f"amax{mt}_{h}")
                        sc16_h = qsmall_pool.tile([128, HB], F16, tag="sc16",
                                                  bufs=4, name=f"sc16{mt}_{h}")
                        r2_h = qsmall_pool.tile([128, HB], F32, tag="r2",
                                                bufs=4, name=f"r2{mt}_{h}")
                        for qq in range(q, q + HQ):
                            xin = xin_pool.tile([128, QC], F32, tag="xin",
                                                name=f"xin{mt}_{qq}")
                            xin_eng.dma_start(
                                out=xin[:],
                                in_=x[mt * 128:(mt + 1) * 128, qq * QC:(qq + 1) * QC])
                            pop_wt_issues(2)
                            nc.vector.tensor_reduce(
                                out=amax_h[:, (qq - q) * NB:(qq - q + 1) * NB],
                                in_=xin.rearrange("p (b c) -> p b c", c=32),
                                axis=mybir.AxisListType.X, op=ALU.max,
                                apply_absolute_value=True)
                            xins[qq] = xin
                        nc.scalar.activation(out=sc16_h[:], in_=amax_h[:],
                                             func=ACT.Copy, scale=float(1.0 / 6.0))
                        nc.vector.reciprocal(out=r2_h[:], in_=sc16_h[:])
                        scale[h] = (sc16_h, r2_h)

                    h = q // HQ
                    qh = q - h * HQ
                    sc16_h, r2_h = scale[h]
                    r2 = r2_h[:, qh * NB:(qh + 1) * NB]
                    sc16 = sc16_h[:, qh * NB:(qh + 1) * NB]
                    xin = xins[q]

                    w = qw_pool.tile([128, QC], F32, tag="qw", name=f"w{mt}_{q}")
                    nc.gpsimd.tensor_tensor(
                        out=w.rearrange("p (b c) -> p b c", c=32),
                        in0=xin.rearrange("p (b c) -> p b c", c=32),
                        in1=r2.unsqueeze(2).broadcast_to([128, NB, 32]),
                        op=ALU.mult)
                    pcb = qpc_pool.tile([128, QC], U32, tag="qpc", name=f"pcb{mt}_{q}")
                    nc.vector.tensor_scalar(out=pcb[:], in0=w[:].bitcast(U32),
                                            scalar1=EXPMASK, scalar2=None,
                                            op0=ALU.bitwise_and)
                    o = qo_pool.tile([128, QC], F32, tag="qo", name=f"o{mt}_{q}")
                    nc.vector.tensor_scalar(out=o[:], in0=pcb[:].bitcast(F32),
                                            scalar1=1.0, scalar2=CR,
                                            op0=ALU.max, op1=ALU.mult)
                    t = qt_pool.tile([128, QC], F32, tag="qt", name=f"t{mt}_{q}")
                    nc.gpsimd.tensor_tensor(out=t[:], in0=w[:], in1=o[:], op=ALU.add)
                    flush_deferred()
                    deferred = (q, t, o, sc16)
                flush_deferred()

            def emit_transposes(mt, qlist):
                for q in qlist:
                    xqc = xqcs[mt][q]
                    dst = xqT[:, mt * Kd + q * QC: mt * Kd + (q + 1) * QC]
                    pt = psumt_pool.tile([128, QC], F16, tag="tp",
                                         name=f"pt{mt}_{q}")
                    for j in range(TPC):
                        nc.tensor.transpose(pt[:, j * 128:(j + 1) * 128],
                                            xqc[:, j * 128:(j + 1) * 128],
                                            ident[:])
                    nc.scalar.copy(out=dst, in_=pt[:])

            # early GEMM MMs (bias pre-injected via K=1 matmul) and their
            # drains are split: drains are emitted TWO m-tiles behind so the
            # ACT copy (which waits on the early MMs) never head-of-line
            # blocks the next m-tile's sc16 / transpose copies on ACT.
            early_psums = {}

            def emit_early_block(mt, b):
                nci, n0, nw, wts, b16row = early[b]
                ps = psum_pool.tile([128, nw], F32, tag="ps", name=f"ps{nci}_{mt}")
                nc.tensor.matmul(out=ps[:], lhsT=ones_row[:], rhs=b16row[:],
                                 start=True, stop=False)
                for k in range(KT):
                    nc.tensor.matmul(out=ps[:], lhsT=lhsT(k, mt), rhs=wts[k][:],
                                     start=False, stop=(k == KT - 1))
                early_psums.setdefault(mt, []).append((nci, n0, nw, ps))

            def emit_early_drains(mt):
                for nci, n0, nw, ps in early_psums.pop(mt):
                    ot = out_pool.tile([128, nw], F32, tag="ot", name=f"ot{nci}_{mt}")
                    nc.scalar.copy(out=ot[:], in_=ps[:])
                    nc.sync.dma_start(out=out[mt * 128:(mt + 1) * 128, n0:n0 + nw],
                                      in_=ot[:])

            # PE order per period: [E(mt-1, b) | T(mt, chunk group b)] so the
            # PE alternates early-GEMM blocks with short transpose bursts and
            # the xqc ring drains steadily.
            ng = QCH // early_nc
            tgroups = [tuple(range(b * ng, QCH if b == early_nc - 1 else (b + 1) * ng))
                       for b in range(early_nc)]
            for mt in range(MT):
                emit_quant(mt)
                # all wt writers must be emitted before their first reader
                pop_wt_issues(len(early_wt_issues))
                if mt >= 2:
                    emit_early_drains(mt - 2)
                if mt == 0:
                    emit_transposes(0, range(QCH))
                else:
                    for b in range(early_nc):
                        emit_early_block(mt - 1, b)
                        emit_transposes(mt, tgroups[b])
            for b in range(early_nc):
                emit_early_block(MT - 1, b)
            emit_early_drains(MT - 2)
            emit_early_drains(MT - 1)

            # ---- Phase B: steady-state GEMM over remaining n-chunks ----
            def load_bias(nci, n0, nw):
                bnc = bias_pool.tile([128, nw], F32, tag="bnc", name=f"bnc{nci}")
                nc.sync.dma_start(
                    out=bnc[:],
                    in_=bias[n0:n0 + nw].unsqueeze(0).broadcast_to([128, nw]))
                return bnc

            for nci in range(early_nc, len(nchunks)):
                n0, nw = nchunks[nci]
                issues, wts = wt_group_issues(nci, n0, nw)
                for iss in issues:
                    iss()
                bnc = load_bias(nci, n0, nw)
                for g in range(0, MT, 4):
                    wave = list(range(g, min(g + 4, MT)))
                    psums = [
                        psum_pool.tile([128, nw], F32, tag="ps", name=f"ps{nci}_{mt}")
                        for mt in wave
                    ]
                    for k in range(KT):
                        for j, mt in enumerate(wave):
                            nc.tensor.matmul(
                                out=psums[j][:], lhsT=lhsT(k, mt), rhs=wts[k][:],
                                start=(k == 0), stop=(k == KT - 1))
                    for j, mt in enumerate(wave):
                        ot = out_pool.tile([128, nw], F32, tag="ot",
                                           name=f"ot{nci}_{mt}")
                        nc.vector.tensor_tensor(out=ot[:], in0=psums[j][:],
                                                in1=bnc[:, :nw], op=ALU.add)
                        nc.sync.dma_start(
                            out=out[mt * 128:(mt + 1) * 128, n0:n0 + nw], in_=ot[:])
    nc.compile()
    return nc


_CACHE = {}


def _get_program():
    if "nc" not in _CACHE:
        _CACHE["nc"] = build_program()
    return _CACHE["nc"]


def run(x, W, bias, trace=False):
    nc = _get_program()
    xf = np.ascontiguousarray(np.asarray(x, dtype=np.float32).reshape(M, K))
    WT16 = np.ascontiguousarray(np.asarray(W, dtype=np.float32).T.astype(np.float16))
    b32 = np.ascontiguousarray(np.asarray(bias, dtype=np.float32))
    in_maps = [
        {"x": xf[c * MS:(c + 1) * MS], "wt": WT16, "bias": b32}
        for c in range(N_CORES)
    ]
    res = run_bass_kernel_spmd(nc, in_maps, list(range(N_CORES)), trace=trace)
    outs = [res.results[c]["out"] for c in range(N_CORES)]
    full = np.concatenate(outs, axis=0).reshape(B, S, N)
    return full, res


def kernel(x, W, bias):
    out, _ = run(x, W, bias, trace=False)
    return out
